# revision 1
# baseline (speedup 1.0000x reference)
"""Data-parallel Trainium kernel for nn_MultiHeadTransformer_18554258719372.

Sharding: batch B=8 -> one sample per NeuronCore (8 cores). All attention /
pooling / normalization axes are local per sample, so no collectives are
needed; weights are replicated. The per-sample forward runs on-device via the
neuron PJRT backend (SPMD across the 8 cores with jax.pmap).
"""

import jax
import jax.numpy as jnp
import numpy as np
from functools import partial

B, CIN, CI, HW, VIEWS, T, V = 8, 256, 128, 64, 4, 4, 17
PART = [[11, 12, 13], [14, 15, 16], [4, 5, 6], [1, 2, 3], [0, 7, 8, 9, 10]]
PERM = [12, 9, 10, 11, 6, 7, 8, 13, 14, 15, 16, 0, 1, 2, 3, 4, 5]


def _attend(xs, Wg, bg, Wt, bt, Wp, bp):
    # xs: (N, Cin, HW, S)
    pool = xs.mean(axis=2)                                        # (N, Cin, S)
    th = jnp.einsum('oc,ncs->nso', Wt, pool) + bt                 # (N, S, Ci)
    ph = jnp.einsum('oc,ncs->nos', Wp, pool) + bp[:, None]        # (N, Ci, S)
    gx = jnp.einsum('oc,nchs->nohs', Wg, xs) + bg[:, None, None]  # (N, Ci, HW, S)
    attn = jax.nn.softmax(jnp.einsum('nqo,nok->nqk', th, ph), axis=-1)
    return jnp.einsum('nqk,nohk->nohq', attn, gx)                 # (N, Ci, HW, S)


def _groupnorm(z, w, b):
    mu = z.mean(axis=(1, 2, 3, 4, 5), keepdims=True)
    var = z.var(axis=(1, 2, 3, 4, 5), keepdims=True)
    zn = (z - mu) * jax.lax.rsqrt(var + 1e-5)
    return zn * w.reshape(1, CIN, HW, 1, 1, 1) + b.reshape(1, CIN, HW, 1, 1, 1)


def _forward(x, g_w, g_b, theta_w, theta_b, phi_w, phi_b, W_w, W_b,
             ln1_w, ln1_b, ff1_w, ff1_b, ffln_w, ffln_b, ff2_w, ff2_b,
             ln2_w, ln2_b):
    # x: (CIN*HW, VIEWS, T, V) -- one sample on this core; add batch dim.
    x = x[None]
    Bx = 1
    x6 = x.reshape(Bx, CIN, HW, VIEWS, T, V)
    # joints head
    xj = x6.transpose(0, 3, 4, 1, 2, 5).reshape(Bx * VIEWS * T, CIN, HW, V)
    yj = _attend(xj, g_w[0], g_b[0], theta_w[0], theta_b[0], phi_w[0], phi_b[0])
    yj = yj.reshape(Bx, VIEWS, T, CI, HW, V).transpose(0, 3, 4, 1, 2, 5)
    # temporal head
    xt = x6.transpose(0, 3, 5, 1, 2, 4).reshape(Bx * VIEWS * V, CIN, HW, T)
    yt = _attend(xt, g_w[1], g_b[1], theta_w[1], theta_b[1], phi_w[1], phi_b[1])
    yt = yt.reshape(Bx, VIEWS, V, CI, HW, T).transpose(0, 3, 4, 1, 5, 2)
    # views head (per body part)
    ys = []
    for p, idx in enumerate(PART):
        Vp = len(idx)
        xp = x6[..., jnp.array(idx)]
        xp = xp.transpose(0, 4, 1, 2, 3, 5).reshape(Bx * T, CIN, HW, VIEWS * Vp)
        yp = _attend(xp, g_w[2 + p], g_b[2 + p], theta_w[2 + p], theta_b[2 + p],
                     phi_w[2 + p], phi_b[2 + p])
        yp = yp.reshape(Bx, T, CI, HW, VIEWS, Vp).transpose(0, 2, 3, 4, 1, 5)
        ys.append(yp)
    yv = jnp.concatenate(ys, axis=-1)[..., jnp.array(PERM)]
    # merge + output projection + residual + GroupNorm
    Y = jnp.concatenate([yj, yt, yv], axis=1)
    z = jnp.einsum('oc,bchvtj->bohvtj', W_w, Y) + W_b.reshape(1, CIN, 1, 1, 1, 1)
    z = _groupnorm(z + x6, ln1_w, ln1_b)
    # feed forward
    h = jnp.einsum('oc,bchvtj->bohvtj', ff1_w, z) + ff1_b.reshape(1, CIN, 1, 1, 1, 1)
    mu = h.mean(axis=(1, 2), keepdims=True)
    var = h.var(axis=(1, 2), keepdims=True)
    h = (h - mu) * jax.lax.rsqrt(var + 1e-5)
    h = h * ffln_w.reshape(1, CIN, HW, 1, 1, 1) + ffln_b.reshape(1, CIN, HW, 1, 1, 1)
    h = jax.nn.relu(h)
    h = jnp.einsum('oc,bchvtj->bohvtj', ff2_w, h) + ff2_b.reshape(1, CIN, 1, 1, 1, 1)
    out = _groupnorm(h + z, ln2_w, ln2_b)
    return out.reshape(CIN * HW, VIEWS, T, V)


_WNAMES = ("g_w", "g_b", "theta_w", "theta_b", "phi_w", "phi_b", "W_w", "W_b",
           "ln1_w", "ln1_b", "ff1_w", "ff1_b", "ffln_w", "ffln_b",
           "ff2_w", "ff2_b", "ln2_w", "ln2_b")

_pmapped = None


def _get_pmapped():
    global _pmapped
    if _pmapped is None:
        # x sharded over batch (axis 0); every weight replicated.
        _pmapped = jax.pmap(
            _forward,
            in_axes=(0,) + (None,) * len(_WNAMES),
            devices=jax.devices()[:8],
        )
    return _pmapped


def kernel(x, **w):
    x = np.asarray(x, dtype=np.float32)
    args = [np.asarray(w[k], dtype=np.float32) for k in _WNAMES]
    fn = _get_pmapped()
    out = fn(x, *args)  # (8, CIN*HW, VIEWS, T, V)
    return np.asarray(out, dtype=np.float32)


if __name__ == "__main__":
    rng = np.random.default_rng(0)
    ins = {"x": rng.standard_normal((B, CIN * HW, VIEWS, T, V), dtype=np.float32)}
    for n, shp in [("g_w", (7, CI, CIN)), ("g_b", (7, CI)),
                   ("theta_w", (7, CI, CIN)), ("theta_b", (7, CI)),
                   ("phi_w", (7, CI, CIN)), ("phi_b", (7, CI)),
                   ("W_w", (CIN, 3 * CI)), ("W_b", (CIN,)),
                   ("ln1_w", (CIN * HW,)), ("ln1_b", (CIN * HW,)),
                   ("ff1_w", (CIN, CIN)), ("ff1_b", (CIN,)),
                   ("ffln_w", (CIN, HW)), ("ffln_b", (CIN,)),
                   ("ff2_w", (CIN, CIN)), ("ff2_b", (CIN,)),
                   ("ln2_w", (CIN * HW,)), ("ln2_b", (CIN * HW,))]:
        ins[n] = (rng.standard_normal(shp, dtype=np.float32) * 0.02
                  if n.endswith("_w") or n in ("g_w", "theta_w", "phi_w", "W_w")
                  else np.zeros(shp, dtype=np.float32))
    out = kernel(**ins)
    print("out", out.shape, out.dtype, float(np.abs(out).max()))
